# revision 58
# baseline (speedup 1.0000x reference)
"""Trainium2 Bass kernel for a GPT transformer block (B=2, T=2048, E=1024, H=16).

Sharding: tensor-parallel over heads for attention (2 heads/core on 8 cores),
4 chunked bf16 AllToAlls redistribute the attention output y from head-sharded
to token-sharded layout (16x less wire than reduce-scattering fp32 proj partial
sums), then each core computes the exact c_proj for its 512 tokens with the
full weight, followed by token-parallel MLP. Host gathers per-core token
slices into the full output.

Layout strategy (per core):
  - LN1 computed in [tok, E] layout (per-token scale/bias native), output cast
    to bf16 and DMA-transposed (xbar) to hT [E, tok] tiles for the qkv matmuls.
    LN gains/biases are folded into the qkv weights on the host.
  - qT/kT computed as [col, tok] (out = W.T @ hT); v computed as vT then
    DMA-transposed back to [tok, col], augmented with a ones column per head so
    the AV matmul also emits softmax denominators (row 64 of each y psum).
  - Attention computed transposed: scoresT[k, q] = kT.T @ qT per head into one
    two-bank psum tile, causal blocks only; one merged exp per k-block on ACT
    (no max subtraction -- scores are bounded by the input distribution);
    normalization via rank-1 outer-product divisor tiles.
  - proj -> fp32 partial sums -> chunked ReduceScatter -> residual -> LN2 ->
    fc/gelu/fc2 (bf16 matmuls, fp32 psum) -> residual -> out.
All matmul operands are bf16 (fp32 matmul runs at 1/4 rate); accumulation,
softmax statistics, LayerNorm and residuals stay fp32. ACT is used only for
Sqrt/Exp/Gelu in contiguous blocks (activation-table locality); psum drains and
affine applies run on DVE.
"""
import sys
if '/opt/trn_rl_repo' not in sys.path:
    sys.path.insert(0, '/opt/trn_rl_repo')

from contextlib import ExitStack

import numpy as np
import ml_dtypes

import concourse.bass as bass
import concourse.bacc as bacc
import concourse.tile as tile
import concourse.mybir as mybir

BF = mybir.dt.bfloat16
F32 = mybir.dt.float32
AF = mybir.ActivationFunctionType
MUL = mybir.AluOpType.mult
ADD = mybir.AluOpType.add

N_CORES = 8
B, T, E, H = 2, 2048, 1024, 16
HD = E // H                      # 64
NTOK = B * T                     # 4096
TOKC = NTOK // N_CORES           # 512 tokens per core for MLP
NCHUNK = 4                       # reduce-scatter chunks (1024 tokens each)
CHTOK = NTOK // NCHUNK           # 1024
SL = CHTOK // N_CORES            # 128 tokens per rank per chunk
EPS = 1e-5
SCALE = 1.0 / np.sqrt(HD)        # 0.125


def build_module(gelu_native=True, debug_taps=False, single_core=False, reps=1):
    nc = bacc.Bacc("TRN2", debug=False, num_devices=1 if single_core else N_CORES)
    dbg = {}

    # ---- I/O ----
    xbf_d = nc.dram_tensor("xbf", [4, 128, 8 * E], BF, kind="ExternalInput")
    xb2_d = nc.dram_tensor("xb2", [TOKC, E], F32, kind="ExternalInput")
    wq_d = nc.dram_tensor("wq", [128, 8, 128], BF, kind="ExternalInput")
    wk_d = nc.dram_tensor("wk", [128, 8, 128], BF, kind="ExternalInput")
    wv_d = nc.dram_tensor("wv", [128, 8, 128], BF, kind="ExternalInput")
    bq_d = nc.dram_tensor("bq", [128, 1], F32, kind="ExternalInput")
    bk_d = nc.dram_tensor("bk", [128, 1], F32, kind="ExternalInput")
    bvb_d = nc.dram_tensor("bvb", [128, 128], F32, kind="ExternalInput")
    wproj_d = nc.dram_tensor("wproj", [128, 8, E], BF, kind="ExternalInput")
    trimask_d = nc.dram_tensor("trimask", [128, 2, 128], BF, kind="ExternalInput")
    wfc_d = nc.dram_tensor("wfc", [8, 128, 4, 8, 128], BF, kind="ExternalInput")
    bfcT_d = nc.dram_tensor("bfcT", [128, 32], F32, kind="ExternalInput")
    wfc2_d = nc.dram_tensor("wfc2", [8, 128, 4, E], BF, kind="ExternalInput")
    bfc2b_d = nc.dram_tensor("bfc2b", [128, E], F32, kind="ExternalInput")
    out_d = nc.dram_tensor("out", [TOKC, E], F32, kind="ExternalOutput")
    if debug_taps:
        for nm, shp in [("dbg_qT", [128, T]), ("dbg_kT", [128, T]),
                        ("dbg_vsb", [128, 16 * 2 * 72]), ("dbg_e01", [128, 1024]),
                        ("dbg_y0", [128, 512]), ("dbg_rec", [65, 1024]),
                        ("dbg_yT", [128, 512]), ("dbg_pp", [128, E]),
                        ("dbg_x2", [128, E]), ("dbg_h4T", [128, 512])]:
            dbg[nm] = nc.dram_tensor(nm, shp, F32, kind="ExternalOutput")

    # internal DRAM for the chunked AllToAll of yT (head-sharded -> token-sharded)
    a2a_ins = [nc.dram_tensor(f"a2a_in{j}", [N_CORES, 128, SL], BF)
               for j in range(NCHUNK)]
    a2a_outs = [nc.dram_tensor(f"a2a_out{j}", [N_CORES, 128, SL], BF)
                for j in range(NCHUNK)]
    RG = [list(range(N_CORES))]

    with ExitStack() as ctx:
        tc = ctx.enter_context(tile.TileContext(nc))

        # ---- persistent pools ----
        # PSUM budget (8 banks): psA tag x2 (1 bank each), s01 x2 (2 banks
        # each), y0 x1, y1 x1.  fc2 reuses: psA x2 + two s01 pairs + y0/y1.
        cst = ctx.enter_context(tc.tile_pool(name="cst", bufs=1))
        psA = ctx.enter_context(tc.tile_pool(name="psA", bufs=2, space="PSUM"))
        psS = ctx.enter_context(tc.tile_pool(name="psS", bufs=2, space="PSUM"))
        psY = ctx.enter_context(tc.tile_pool(name="psY", bufs=1, space="PSUM"))
        ppp = ctx.enter_context(tc.tile_pool(name="ppp", bufs=3))

        # ---- constants ----
        wq_sb = cst.tile([128, 8, 128], BF, tag="wq")
        wk_sb = cst.tile([128, 8, 128], BF, tag="wk")
        wv_sb = cst.tile([128, 8, 128], BF, tag="wv")
        nc.scalar.dma_start(wq_sb[:], wq_d[:, :, :])
        nc.scalar.dma_start(wk_sb[:], wk_d[:, :, :])
        nc.scalar.dma_start(wv_sb[:], wv_d[:, :, :])
        bq_sb = cst.tile([128, 1], F32, tag="bq")
        bk_sb = cst.tile([128, 1], F32, tag="bk")
        bvb_sb = cst.tile([128, 128], F32, tag="bvb")
        nc.scalar.dma_start(bq_sb[:], bq_d[:, :])
        nc.scalar.dma_start(bk_sb[:], bk_d[:, :])
        nc.scalar.dma_start(bvb_sb[:], bvb_d[:, :])
        wproj_sb = cst.tile([128, 8, E], BF, tag="wproj")
        trimask_sb = cst.tile([128, 2, 128], BF, tag="trimask")
        nc.scalar.dma_start(trimask_sb[:], trimask_d[:, :, :])
        ones_row = cst.tile([65, 64], BF, tag="ones_row")   # D outer-product lhsT
        nc.gpsimd.memset(ones_row[:], 1.0)
        eps_sb = cst.tile([128, 1], F32, tag="eps")
        nc.gpsimd.memset(eps_sb[:], EPS)

        def tap(name, ap, pool_):
            if not debug_taps or name not in dbg:
                return
            t = pool_.tile([ap.shape[0], int(np.prod(ap.shape[1:]))], F32,
                           tag="dbgt", name=f"tap_{name}")
            nc.vector.tensor_copy(t[:ap.shape[0], :], ap)
            nc.sync.dma_start(dbg[name][0:ap.shape[0], :], t[:ap.shape[0], :])
            dbg.pop(name)

        wproj_loaded = [False]

        def emit_body():
            with ExitStack() as p1:
                xp = p1.enter_context(tc.tile_pool(name="xp", bufs=3))
                stp = p1.enter_context(tc.tile_pool(name="stp", bufs=4))
                hp = p1.enter_context(tc.tile_pool(name="hp", bufs=3))
                hTp = p1.enter_context(tc.tile_pool(name="hTp", bufs=1))
                qkp = p1.enter_context(tc.tile_pool(name="qkp", bufs=2))
                vp = p1.enter_context(tc.tile_pool(name="vp", bufs=2))
                expp = p1.enter_context(tc.tile_pool(name="expp", bufs=4))
                srp = p1.enter_context(tc.tile_pool(name="srp", bufs=2))
                yTp = p1.enter_context(tc.tile_pool(name="yTp", bufs=2))
                ytmp = p1.enter_context(tc.tile_pool(name="ytmp", bufs=2))
                dbgp = p1.enter_context(tc.tile_pool(name="dbgp", bufs=1)) if debug_taps else None

                # ---------- phase A: LN1 + transpose to hT (both batches) ----------
                hTs = []
                for b in range(B):
                    base = b * T
                    # per-512-token-chunk tiles so consumers start as soon as
                    # their chunk is transposed (Tile deps are whole-tile)
                    hT = [hTp.tile([128, 8, 512], BF, tag=f"hT{b}_{ch}",
                                   name=f"hT{b}_{ch}") for ch in range(4)]
                    hTs.append(hT)
                    for half in range(2):
                        xbig = xp.tile([128, 8, E], BF, tag="xbig")
                        nc.sync.dma_start(
                            xbig[:].rearrange("p g e -> p (g e)"),
                            xbf_d[b * 2 + half, :, :])
                        # stats pass: one grouped sqrt per 4 tiles keeps ACT
                        # mostly free for the attention exps without delaying
                        # the first hT tiles too long
                        for q4 in range(2):
                            mv = stp.tile([128, 4, 2], F32, tag="mv")
                            for gi in range(4):
                                g = q4 * 4 + gi
                                st = stp.tile([128, 2, 6], F32, tag="st")
                                nc.vector.bn_stats(st[:, 0, :], xbig[:, g, 0:512])
                                nc.vector.bn_stats(st[:, 1, :], xbig[:, g, 512:1024])
                                nc.vector.bn_aggr(mv[:, gi, :], st[:])
                            rstd = stp.tile([128, 4], F32, tag="rstd")
                            nc.scalar.activation(rstd[:], mv[:, :, 1], AF.Sqrt, bias=eps_sb[:])
                            nc.vector.reciprocal(rstd[:], rstd[:])
                            nmr = stp.tile([128, 4], F32, tag="nmr")
                            nc.vector.tensor_mul(nmr[:], mv[:, :, 0], rstd[:])
                            for gi in range(4):
                                g = q4 * 4 + gi
                                tt = half * 8 + g
                                h_sb = hp.tile([128, E], BF, tag="h")
                                nc.vector.tensor_scalar(h_sb[:], xbig[:, g, :],
                                                        rstd[:, gi:gi + 1], nmr[:, gi:gi + 1],
                                                        op0=MUL, op1=mybir.AluOpType.subtract)
                                nc.sync.dma_start_transpose(
                                    hT[tt // 4][:, :, (tt % 4) * 128:(tt % 4 + 1) * 128],
                                    h_sb[:])

                # ---------- phase B: qkv for both batches, then attention ----------
                qTs, kTs, vs = [], [], []
                for b in range(B):
                    hT = hTs[b]
                    qT = [qkp.tile([128, 512], BF, tag=f"qT{ch}", name=f"qT{b}_{ch}")
                          for ch in range(4)]
                    kT = [qkp.tile([128, 512], BF, tag=f"kT{ch}", name=f"kT{b}_{ch}")
                          for ch in range(4)]
                    qTs.append(qT)
                    kTs.append(kT)
                    # v computed directly in [tok, col] layout (stationary = hT
                    # slice): per-chunk tiles [128, 4 kb, head, 72]: v cols 0:64,
                    # ones col at 64 (its own 16-byte SBUF line via the stride).
                    v_sb = [vp.tile([128, 4, 2, 72], BF, tag=f"v{ch}",
                                    name=f"v{b}_{ch}") for ch in range(4)]
                    vs.append(v_sb)
                    for ch in range(4):
                        for w_sb, b_sb, dst in ((wq_sb, bq_sb, qT), (wk_sb, bk_sb, kT)):
                            ps = psA.tile([128, 512], F32, tag="psA", name="qkv_ps")
                            for e in range(8):
                                nc.tensor.matmul(ps[:], w_sb[:, e, :],
                                                 hT[ch][:, e, :],
                                                 start=(e == 0), stop=(e == 7))
                            nc.vector.tensor_scalar_add(dst[ch][:], ps[:], b_sb[:])
                        nc.gpsimd.memset(v_sb[ch][:, :, :, 64:65], 1.0)
                        for ti in range(4):
                            ps = psA.tile([128, 512], F32, tag="psA", name="v_ps")
                            for e in range(8):
                                nc.tensor.matmul(ps[:, 0:128],
                                                 hT[ch][:, e, ti * 128:(ti + 1) * 128],
                                                 wv_sb[:, e, :], start=(e == 0), stop=(e == 7))
                            nc.vector.tensor_add(
                                v_sb[ch][:, ti, :, 0:64],
                                ps[:, 0:128].rearrange("p (g x) -> p g x", g=2),
                                bvb_sb[:].rearrange("p (g x) -> p g x", g=2))
                if not wproj_loaded[0]:
                    wproj_loaded[0] = True
                    nc.gpsimd.dma_start(wproj_sb[:], wproj_d[:, :, :])
                for b in range(B):
                    base = b * T
                    qT, kT, v_sb = qTs[b], kTs[b], vs[b]


                    for qc in range(T // 512):
                        qs = qc * 512
                        y0 = psY.tile([128, 512], F32, tag="y0")
                        y1 = psY.tile([128, 512], F32, tag="y1")
                        nkb = 4 * qc + 4
                        # software-pipelined: emit scores/exp LOOKAHEAD k-blocks
                        # ahead of the AV consumers, so the in-order PE queue
                        # never stalls an AV matmul behind the exp it needs.
                        LOOKAHEAD = 3
                        e01s = {}
                        for k in range(nkb + LOOKAHEAD):
                            if k < nkb:
                                kb = k
                                d = max(0, kb * 128 - qs)
                                kc, ki = kb // 4, (kb % 4) * 128
                                s01 = psS.tile([128, 2, 512], F32, tag="s01")
                                nc.tensor.matmul(s01[:, 0, d:512],
                                                 kT[kc][0:64, ki:ki + 128],
                                                 qT[qc][0:64, d:512])
                                nc.tensor.matmul(s01[:, 1, d:512],
                                                 kT[kc][64:128, ki:ki + 128],
                                                 qT[qc][64:128, d:512])
                                e01 = expp.tile([128, 2, 512], BF, tag="e01")
                                nc.scalar.activation(e01[:, :, d:512], s01[:, :, d:512],
                                                     AF.Exp, scale=SCALE)
                                if kb * 128 >= qs:
                                    nc.vector.tensor_mul(e01[:, :, d:d + 128],
                                                         e01[:, :, d:d + 128], trimask_sb[:])
                                e01s[kb] = e01
                            if k >= LOOKAHEAD:
                                kb = k - LOOKAHEAD
                                d = max(0, kb * 128 - qs)
                                kc = kb // 4
                                e01 = e01s.pop(kb)
                                st_, sp_ = (kb == 0), (kb == nkb - 1)
                                nc.tensor.matmul(y0[0:65, d:512], v_sb[kc][:, kb % 4, 0, 0:65],
                                                 e01[:, 0, d:512], start=st_, stop=sp_)
                                nc.tensor.matmul(y1[0:65, d:512], v_sb[kc][:, kb % 4, 1, 0:65],
                                                 e01[:, 1, d:512], start=st_, stop=sp_)
                        # drain y psums early (releases psY for the next qc),
                        # then normalize: reciprocal denominators (row 64) ->
                        # rank-1 divisor tiles in psum -> bf16 yT blocks.
                        if b == 0 and qc == 0 and debug_taps:
                            tap("dbg_y0", y0[:], dbgp)
                        ysb = srp.tile([65, 2, 512], F32, tag="ysb")
                        recb = srp.tile([65, 2, 512], BF, tag="recb")
                        nc.vector.tensor_copy(ysb[0:65, 0, :], y0[0:65, :])
                        nc.vector.tensor_copy(ysb[0:65, 1, :], y1[0:65, :])
                        nc.vector.reciprocal(ysb[64:65, :, :], ysb[64:65, :, :])
                        nc.vector.tensor_copy(recb[64:65, :, :], ysb[64:65, :, :])
                        D0 = psA.tile([128, 512], F32, tag="psA", name="D0")
                        D1 = psA.tile([128, 512], F32, tag="psA", name="D1")
                        nc.tensor.matmul(D0[0:64, :], ones_row[64:65, :], recb[64:65, 0, :],
                                         tile_position=(64, 0))
                        nc.tensor.matmul(D1[0:64, :], ones_row[64:65, :], recb[64:65, 1, :],
                                         tile_position=(64, 0))
                        if b == 0 and qc == 0 and debug_taps:
                            tap("dbg_rec", ysb[:].rearrange("p a b -> p (a b)"), dbgp)
                        yT = yTp.tile([64, 4, 128], BF, tag="yT")
                        y1t = ytmp.tile([64, 4, 128], BF, tag="y1t")
                        nc.vector.tensor_mul(yT[:].rearrange("p m t -> p (m t)"),
                                             ysb[0:64, 0, :], D0[0:64, :])
                        nc.vector.tensor_mul(y1t[:].rearrange("p m t -> p (m t)"),
                                             ysb[0:64, 1, :], D1[0:64, :])
                        if b == 0 and qc == 0 and debug_taps:
                            tap("dbg_yT", yT[:].rearrange("p m t -> p (m t)"), dbgp)
                        # ---------- scatter yT blocks for the AllToAll ----------
                        j = b * 2 + qc // 2
                        p4 = (qc % 2) * 4
                        nc.gpsimd.dma_start(
                            a2a_ins[j][p4:p4 + 4, 0:64, :].rearrange("m p t -> p m t"),
                            yT[:])
                        nc.gpsimd.dma_start(
                            a2a_ins[j][p4:p4 + 4, 64:128, :].rearrange("m p t -> p m t"),
                            y1t[:])
                        # fire the all-to-all for each completed 1024-token chunk
                        if qc % 2 == 1:
                            if single_core:
                                t_ = ppp.tile([128, N_CORES, SL], BF, tag="a2afake")
                                nc.sync.dma_start(
                                    t_[:], a2a_ins[j][:, :, :].rearrange("s p t -> p s t"))
                                nc.sync.dma_start(
                                    a2a_outs[j][:, :, :].rearrange("s p t -> p s t"), t_[:])
                            else:
                                nc.gpsimd.collective_compute(
                                    "AllToAll", mybir.AluOpType.bypass,
                                    replica_groups=RG,
                                    ins=[a2a_ins[j][:, :, :]],
                                    outs=[a2a_outs[j][:, :, :]])

            # ---------- phase C: MLP (token-parallel, full weights) ----------
            with ExitStack() as p2:
                x2p = p2.enter_context(tc.tile_pool(name="x2p", bufs=1))
                st2 = p2.enter_context(tc.tile_pool(name="st2", bufs=4))
                h2p = p2.enter_context(tc.tile_pool(name="h2p", bufs=2))
                h2Tp = p2.enter_context(tc.tile_pool(name="h2Tp", bufs=1))
                h4Tp = p2.enter_context(tc.tile_pool(name="h4Tp", bufs=1))
                wfcp = p2.enter_context(tc.tile_pool(name="wfcp", bufs=3))
                wf2p = p2.enter_context(tc.tile_pool(name="wf2p", bufs=3))
                outp = p2.enter_context(tc.tile_pool(name="outp", bufs=2))
                gwp = p2.enter_context(tc.tile_pool(name="gwp", bufs=2))
                dbgp2 = p2.enter_context(tc.tile_pool(name="dbgp2", bufs=2)) if debug_taps else None
                cst2 = p2.enter_context(tc.tile_pool(name="cst2", bufs=1))

                bfcT_sb = cst2.tile([128, 32], F32, tag="bfcT")
                nc.sync.dma_start(bfcT_sb[:], bfcT_d[:, :])
                bfc2b_sb = cst2.tile([128, E], F32, tag="bfc2b")
                nc.sync.dma_start(bfc2b_sb[:], bfc2b_d[:, :])

                h2T = h2Tp.tile([128, 8, TOKC], BF, tag="h2T")
                mv4 = st2.tile([128, 4, 2], F32, tag="mv4")
                x2_tiles = []
                for mt in range(4):
                    ycc = x2p.tile([128, 8, SL], BF, tag=f"ycc_{mt}", name=f"ycc_{mt}")
                    nc.gpsimd.dma_start(
                        ycc[:], a2a_outs[mt][:, :, :].rearrange("s p t -> p s t"))
                    pp0 = psA.tile([128, 512], F32, tag="psA", name="pp0")
                    pp1 = psA.tile([128, 512], F32, tag="psA", name="pp1")
                    for c in range(8):
                        nc.tensor.matmul(pp0[:], ycc[:, c, :], wproj_sb[:, c, 0:512],
                                         start=(c == 0), stop=(c == 7))
                        nc.tensor.matmul(pp1[:], ycc[:, c, :], wproj_sb[:, c, 512:1024],
                                         start=(c == 0), stop=(c == 7))
                    x2 = x2p.tile([128, E], F32, tag=f"x2_{mt}", name=f"x2_{mt}")
                    x2_tiles.append(x2)
                    xr = x2p.tile([128, E], F32, tag=f"xr_{mt}")
                    nc.sync.dma_start(xr[:], xb2_d[mt * 128:(mt + 1) * 128, :])
                    nc.vector.tensor_add(x2[:, 0:512], pp0[:], xr[:, 0:512])
                    nc.vector.tensor_add(x2[:, 512:1024], pp1[:], xr[:, 512:1024])
                    st = st2.tile([128, 2, 6], F32, tag="st")
                    nc.vector.bn_stats(st[:, 0, :], x2[:, 0:512])
                    nc.vector.bn_stats(st[:, 1, :], x2[:, 512:1024])
                    nc.vector.bn_aggr(mv4[:, mt, :], st[:])

                # single LN2 sqrt after the last exp: fc needs all four h2T
                # chunks anyway, and this avoids ACT table thrash mid-stream
                rstd4 = st2.tile([128, 4], F32, tag="rstd4")
                nc.scalar.activation(rstd4[:], mv4[:, :, 1], AF.Sqrt, bias=eps_sb[:])
                nc.vector.reciprocal(rstd4[:], rstd4[:])
                nmr4 = st2.tile([128, 4], F32, tag="nmr4")
                nc.vector.tensor_mul(nmr4[:], mv4[:, :, 0], rstd4[:])
                for mt in range(4):
                    h2 = h2p.tile([128, E], BF, tag="h2")
                    nc.vector.tensor_scalar(h2[:], x2_tiles[mt][:],
                                            rstd4[:, mt:mt + 1], nmr4[:, mt:mt + 1],
                                            op0=MUL, op1=mybir.AluOpType.subtract)
                    nc.scalar.dma_start_transpose(
                        h2T[:, :, mt * 128:(mt + 1) * 128], h2[:])

                if debug_taps:
                    tap("dbg_x2", x2_tiles[0][:], dbgp2)
                # fc + gelu -> h4T
                h4T = h4Tp.tile([128, 32, TOKC], BF, tag="h4T")
                for m in range(32):
                    if m % 4 == 0:
                        wfc4 = wfcp.tile([128, 4, 8, 128], BF, tag="wfc")
                        nc.sync.dma_start(wfc4[:], wfc_d[m // 4, :, :, :, :])
                    wfc_sb = wfc4[:, m % 4, :, :]
                    h3 = psA.tile([128, 512], F32, tag="psA", name="h3")
                    for e in range(8):
                        nc.tensor.matmul(h3[:], wfc_sb[:, e, :], h2T[:, e, :],
                                         start=(e == 0), stop=(e == 7))
                    if gelu_native:
                        nc.scalar.activation(h4T[:, m, :], h3[:], AF.Gelu_apprx_tanh,
                                             bias=bfcT_sb[:, m:m + 1])
                    else:
                        # tanh-gelu built from sim-supported ops:
                        #   u = c*(h3b + 0.044715*h3b^3); h4 = h3b*(0.5+0.5*tanh(u))
                        h3b = gwp.tile([128, 512], F32, tag="h3b")
                        nc.vector.tensor_scalar_add(h3b[:], h3[:], bfcT_sb[:, m:m + 1])
                        sq = gwp.tile([128, 512], F32, tag="sq")
                        nc.vector.tensor_mul(sq[:], h3b[:], h3b[:])
                        nc.vector.tensor_scalar(sq[:], sq[:], 0.044715, 1.0,
                                                op0=MUL, op1=ADD)
                        nc.vector.tensor_mul(sq[:], sq[:], h3b[:])
                        th = gwp.tile([128, 512], F32, tag="th")
                        nc.scalar.activation(th[:], sq[:], AF.Tanh,
                                             scale=float(np.sqrt(2.0 / np.pi)))
                        nc.vector.tensor_scalar(th[:], th[:], 0.5, 0.5, op0=MUL, op1=ADD)
                        nc.vector.tensor_mul(h4T[:, m, :], th[:], h3b[:])

                if debug_taps:
                    tap("dbg_h4T", h4T[:, 0, :], dbgp2)
                # fc2: all 8 psum banks accumulate across the single weight stream
                y2 = [
                    (psA.tile([128, 512], F32, tag="psA", name="y2a0"),
                     psA.tile([128, 512], F32, tag="psA", name="y2b0")),
                    (psS.tile([128, 2, 512], F32, tag="s01", name="y2p1"),),
                    (psS.tile([128, 2, 512], F32, tag="s01", name="y2p2"),),
                    (psY.tile([128, 512], F32, tag="y0", name="y2a3"),
                     psY.tile([128, 512], F32, tag="y1", name="y2b3")),
                ]

                def y2ap(mt, half):
                    tt = y2[mt]
                    if len(tt) == 1:
                        return tt[0][:, half, :]
                    return tt[half][:]

                # pre-add the fc2 bias into the residual while PE is busy, so
                # the post-fc2 drain is a single add per half
                for mt in range(4):
                    nc.vector.tensor_add(x2_tiles[mt][:], x2_tiles[mt][:], bfc2b_sb[:])
                for k in range(32):
                    if k % 4 == 0:
                        wf24 = wf2p.tile([128, 4, E], BF, tag="wf2")
                        nc.sync.dma_start(wf24[:], wfc2_d[k // 4, :, :, :])
                    wf2 = wf24[:, k % 4, :]
                    for mt in range(4):
                        nc.tensor.matmul(y2ap(mt, 0), h4T[:, k, mt * 128:(mt + 1) * 128],
                                         wf2[:, 0:512], start=(k == 0), stop=(k == 31))
                        nc.tensor.matmul(y2ap(mt, 1), h4T[:, k, mt * 128:(mt + 1) * 128],
                                         wf2[:, 512:1024], start=(k == 0), stop=(k == 31))
                for mt in range(4):
                    o = outp.tile([128, E], F32, tag="o")
                    nc.vector.tensor_add(o[:, 0:512], y2ap(mt, 0), x2_tiles[mt][:, 0:512])
                    nc.vector.tensor_add(o[:, 512:1024], y2ap(mt, 1), x2_tiles[mt][:, 512:1024])
                    nc.gpsimd.dma_start(out_d[mt * 128:(mt + 1) * 128, :], o[:])


        for _rep in range(reps):
            emit_body()

    nc.compile()
    return nc


def prep_inputs(x, ln1_g, ln1_b, w_attn, b_attn, w_proj, b_proj,
                ln2_g, ln2_b, w_fc, b_fc, w_fc2, b_fc2):
    """Host-side prep: fold LN affine into weights, slice per core, cast bf16."""
    bf16 = ml_dtypes.bfloat16
    x_flat = np.asarray(x, np.float32).reshape(NTOK, E)
    w_attn = np.asarray(w_attn, np.float32)
    ln1_g = np.asarray(ln1_g, np.float32)
    ln1_b = np.asarray(ln1_b, np.float32)
    ln2_g = np.asarray(ln2_g, np.float32)
    ln2_b = np.asarray(ln2_b, np.float32)
    wa_eff = ln1_g[:, None] * w_attn
    ba_eff = ln1_b @ w_attn + np.asarray(b_attn, np.float32)
    wf_eff = ln2_g[:, None] * np.asarray(w_fc, np.float32)
    bf_eff = ln2_b @ np.asarray(w_fc, np.float32) + np.asarray(b_fc, np.float32)

    tri = np.broadcast_to(
        (np.arange(128)[None, None, :] >= np.arange(128)[:, None, None]),
        (128, 2, 128)).astype(bf16)
    wproj_arr = np.ascontiguousarray(
        np.asarray(w_proj, np.float32).reshape(8, 128, E).transpose(1, 0, 2)).astype(bf16)
    # x prearranged into contiguous per-partition tile layout:
    # xbf4[i, p, g*E+e] = x[i*1024 + g*128 + p, e]
    xbf4 = np.ascontiguousarray(
        x_flat.reshape(4, 8, 128, E).transpose(0, 2, 1, 3).reshape(4, 128, 8 * E)
    ).astype(bf16)
    # wfc[m4, p, mi, g, c] = wf_eff[g*128+p, (m4*4+mi)*128+c]
    wfc_arr = np.ascontiguousarray(
        wf_eff.reshape(8, 128, 8, 4, 128).transpose(2, 1, 3, 0, 4)).astype(bf16)
    bfcT = bf_eff.reshape(32, 128).T.astype(np.float32).copy()
    # wfc2[k4, p, ki, e] = w_fc2[(k4*4+ki)*128+p, e]
    wfc2_arr = np.ascontiguousarray(
        np.asarray(w_fc2, np.float32).reshape(8, 4, 128, E).transpose(0, 2, 1, 3)).astype(bf16)
    bfc2b = np.broadcast_to(np.asarray(b_fc2, np.float32), (128, E)).copy()

    in_maps = []
    for r in range(N_CORES):
        qc_ = slice(128 * r, 128 * r + 128)
        kc_ = slice(E + 128 * r, E + 128 * r + 128)
        vc_ = slice(2 * E + 128 * r, 2 * E + 128 * r + 128)
        xb2 = np.concatenate(
            [x_flat[j * CHTOK + r * SL: j * CHTOK + (r + 1) * SL] for j in range(NCHUNK)],
            axis=0) + np.asarray(b_proj, np.float32)
        in_maps.append({
            "xbf": xbf4,
            "xb2": xb2.astype(np.float32),
            "wq": np.ascontiguousarray(wa_eff[:, qc_].reshape(8, 128, 128).transpose(1, 0, 2)).astype(bf16),
            "wk": np.ascontiguousarray(wa_eff[:, kc_].reshape(8, 128, 128).transpose(1, 0, 2)).astype(bf16),
            "wv": np.ascontiguousarray(wa_eff[:, vc_].reshape(8, 128, 128).transpose(1, 0, 2)).astype(bf16),
            "bq": ba_eff[qc_].reshape(128, 1).astype(np.float32),
            "bk": ba_eff[kc_].reshape(128, 1).astype(np.float32),
            "bvb": np.tile(ba_eff[vc_].astype(np.float32), (128, 1)),
            "wproj": wproj_arr,
            "trimask": tri,
            "wfc": wfc_arr,
            "bfcT": bfcT,
            "wfc2": wfc2_arr,
            "bfc2b": bfc2b,
        })
    return in_maps


def gather_output(results):
    out_flat = np.empty((NTOK, E), np.float32)
    for r in range(N_CORES):
        o = results[r]["out"]
        for j in range(NCHUNK):
            out_flat[j * CHTOK + r * SL: j * CHTOK + (r + 1) * SL] = o[j * SL:(j + 1) * SL]
    return out_flat.reshape(B, T, E)


_CACHE = {}


def _get_runner():
    if "runner" in _CACHE:
        return _CACHE["runner"]
    import jax
    from jax.sharding import Mesh, PartitionSpec, NamedSharding
    from jax.experimental.shard_map import shard_map
    from concourse.bass2jax import _bass_exec_p, install_neuronx_cc_hook, partition_id_tensor

    nc = build_module()
    install_neuronx_cc_hook()
    partition_name = nc.partition_id_tensor.name if nc.partition_id_tensor else None
    in_names, out_names, out_avals = [], [], []
    for alloc in nc.m.functions[0].allocations:
        if not isinstance(alloc, mybir.MemoryLocationSet):
            continue
        name = alloc.memorylocations[0].name
        if alloc.kind == "ExternalInput":
            if name != partition_name:
                in_names.append(name)
        elif alloc.kind == "ExternalOutput":
            out_names.append(name)
            out_avals.append(jax.core.ShapedArray(
                tuple(alloc.tensor_shape), mybir.dt.np(alloc.dtype)))
    all_in = in_names + out_names + ([partition_name] if partition_name else [])

    def _body(*args):
        operands = list(args)
        if partition_name is not None:
            operands.append(partition_id_tensor())
        return tuple(_bass_exec_p.bind(
            *operands, out_avals=tuple(out_avals), in_names=tuple(all_in),
            out_names=tuple(out_names), lowering_input_output_aliases=(),
            sim_require_finite=True, sim_require_nnan=True, nc=nc))

    devices = jax.devices()[:N_CORES]
    mesh = Mesh(np.asarray(devices), ("core",))
    n_io = len(in_names) + len(out_names)
    fn = jax.jit(
        shard_map(_body, mesh=mesh, in_specs=(PartitionSpec("core"),) * n_io,
                  out_specs=(PartitionSpec("core"),) * len(out_names),
                  check_rep=False),
        keep_unused=True)
    sharding = NamedSharding(mesh, PartitionSpec("core"))
    _CACHE["runner"] = (fn, in_names, out_names, out_avals, sharding)
    return _CACHE["runner"]


def run_device(in_maps):
    import jax
    fn, in_names, out_names, out_avals, sharding = _get_runner()
    concat_in = [
        np.concatenate([np.asarray(in_maps[c][n]) for c in range(N_CORES)], axis=0)
        for n in in_names]
    concat_zero = [np.zeros((N_CORES * a.shape[0], *a.shape[1:]), a.dtype)
                   for a in out_avals]
    args = [jax.device_put(a, sharding) for a in concat_in + concat_zero]
    outs = fn(*args)
    jax.block_until_ready(outs)
    return [
        {n: np.asarray(outs[i]).reshape(N_CORES, *out_avals[i].shape)[c]
         for i, n in enumerate(out_names)}
        for c in range(N_CORES)], args, fn


def kernel(**inputs):
    in_maps = prep_inputs(**inputs)
    results, _, _ = run_device(in_maps)
    return gather_output(results).astype(np.float32)

